# revision 27
# baseline (speedup 1.0000x reference)
"""Causal self-attention (B=2, T=2048, D=1024, H=16, rope) on 8 Trainium2 cores.

Sharding: heads are split across cores (2 heads/core, tensor-parallel):
each core computes QKV projection columns for its heads, RoPE, causal
attention, and a partial out-projection (its rows of w_out). The host sums
the 8 partial outputs (the tensor-parallel all-reduce, done at gather time).

All matmul operands are fp16 (fp32 PSUM accumulation). Activations flow
feature-major (transposed) so every matmul contracts along the partition
dim; the host transposes x on the way in and the output back on the way
out. Softmax denominators come free from a ones-column appended to V;
exp runs biased (exp(s/8 - 4)) to stay inside fp16 range, the bias cancels
in the normalization.
"""

import sys

for _p in ("/opt/trn_rl_repo",):
    if _p not in sys.path:
        sys.path.insert(0, _p)

import numpy as np

B, T, D, H = 2, 2048, 1024, 16
DH = D // H  # 64
N_CORES = 8
HPC = H // N_CORES  # heads per core = 2
BT = B * T  # 4096
ROPE_BASE = 10000.0
EXP_BIAS = -4.0

_CACHE = {}


def _host_consts():
    # RoPE tables, feature-major, two heads stacked: [128, T]
    inv_freq = 1.0 / (ROPE_BASE ** (np.arange(0, DH, 2, dtype=np.float32) / DH))
    t = np.arange(T, dtype=np.float32)
    freqs = np.outer(t, inv_freq)  # [T, 32]
    emb = np.concatenate([freqs, freqs], axis=-1)  # [T, 64]
    cosT = np.cos(emb).T.astype(np.float32)  # [64, T]
    sinT = np.sin(emb).T.astype(np.float32)
    # sign baked for the rotate-half term: rows 0:32 get -sin, rows 32:64 +sin
    sinS = np.concatenate([-sinT[:32], sinT[32:]], axis=0)
    cosb = np.concatenate([cosT, cosT], axis=0).astype(np.float16)
    sinb = np.concatenate([sinS, sinS], axis=0).astype(np.float16)
    # Causal masks for the 4 diagonal-block offsets o = 0,128,256,384,
    # concatenated along free dim: [128, 2048]
    p = np.arange(128)[:, None]
    f = np.arange(512)[None, :]
    mask = np.zeros((128, 4 * 512), dtype=np.float16)
    for tno in range(4):
        o = 128 * tno
        mask[:, tno * 512:(tno + 1) * 512] = (f >= o + p).astype(np.float16)
    return cosb, sinb, mask


def _build(debug=False):
    """Build + schedule the per-core Bass module (same program on all cores)."""
    from concourse import bacc
    import concourse.mybir as mybir
    import concourse.tile as tile

    F16 = mybir.dt.float16
    F32 = mybir.dt.float32
    AF = mybir.ActivationFunctionType

    nc = bacc.Bacc("TRN2", target_bir_lowering=False, debug=False,
                   num_devices=N_CORES)

    xt_d = nc.dram_tensor("xt", [D, BT], F16, kind="ExternalInput")
    wq_d = nc.dram_tensor("wq", [D, 128], F16, kind="ExternalInput")
    wk_d = nc.dram_tensor("wk", [D, 128], F16, kind="ExternalInput")
    wv_d = nc.dram_tensor("wv", [D, 128], F16, kind="ExternalInput")
    wo_d = nc.dram_tensor("wo", [128, D], F16, kind="ExternalInput")
    cos_d = nc.dram_tensor("cosb", [128, T], F16, kind="ExternalInput")
    sin_d = nc.dram_tensor("sinb", [128, T], F16, kind="ExternalInput")
    mask_d = nc.dram_tensor("mask", [128, 2048], F16, kind="ExternalInput")
    out_d = nc.dram_tensor("outp", [D, BT], F32, kind="ExternalOutput")
    if debug:
        qt_o = nc.dram_tensor("qt_o", [128, BT], F16, kind="ExternalOutput")
        kt_o = nc.dram_tensor("kt_o", [128, BT], F16, kind="ExternalOutput")
        vt_o = nc.dram_tensor("vt_o", [128, BT], F16, kind="ExternalOutput")
        on_o = nc.dram_tensor("on_o", [128, BT], F16, kind="ExternalOutput")
        vp_o = nc.dram_tensor("vp_o", [128, 16 * 130], F16, kind="ExternalOutput")

    NK = D // 128       # 8 contraction chunks for qkv projection
    NS = BT // 512      # 8 token slices
    NJ = T // 512       # 4 tq slices per batch
    NB = T // 128       # 16 tk blocks per batch
    PIPE = 2            # exp->AV software pipeline depth, in 2-block groups

    with tile.TileContext(nc) as tc:
        with (
            tc.tile_pool(name="consts", bufs=1) as consts,
            tc.tile_pool(name="acts", bufs=1) as acts,
        ):
            wq = consts.tile([128, NK, 128], F16)
            wk = consts.tile([128, NK, 128], F16)
            wv = consts.tile([128, NK, 128], F16)
            wo = consts.tile([128, NK, 128], F16)
            nc.sync.dma_start(out=wq, in_=wq_d[:, :].rearrange("(k p) f -> p k f", p=128))
            nc.sync.dma_start(out=wk, in_=wk_d[:, :].rearrange("(k p) f -> p k f", p=128))
            nc.sync.dma_start(out=wv, in_=wv_d[:, :].rearrange("(k p) f -> p k f", p=128))
            nc.sync.dma_start(out=wo, in_=wo_d[:, :].rearrange("p (m f) -> p m f", m=NK))
            cosb = consts.tile([128, T], F16)
            sinb = consts.tile([128, T], F16)
            mask = consts.tile([128, 2048], F16)
            nc.sync.dma_start(out=cosb, in_=cos_d[:, :])
            nc.sync.dma_start(out=sinb, in_=sin_d[:, :])
            nc.sync.dma_start(out=mask, in_=mask_d[:, :])
            ones16 = consts.tile([128, NB], F16)
            nc.vector.memset(ones16, 1.0)
            ebias = consts.tile([128, 1], F32)
            nc.vector.memset(ebias, EXP_BIAS)

            qt = acts.tile([128, BT], F16)  # rows: [h0 d0..63 | h1 d0..63]
            kt = acts.tile([128, BT], F16)
            vt = acts.tile([128, BT], F16)

            # ---------------- Phase 1: QKV^T projection + RoPE ----------
            with (
                tc.tile_pool(name="xt", bufs=2) as xtp,
                tc.tile_pool(name="rope", bufs=2) as rope,
                tc.tile_pool(name="qkv_ps", bufs=6, space="PSUM") as qkv_ps,
            ):
                xt_r = xt_d[:, :].rearrange("(k p) t -> p k t", p=128)
                for n in range(NS):
                    ts = slice(n * 512, (n + 1) * 512)
                    cs = slice((n % NJ) * 512, (n % NJ) * 512 + 512)
                    xtt = xtp.tile([128, NK, 512], F16, tag="xt")
                    nc.sync.dma_start(out=xtt, in_=xt_r[:, :, ts])
                    pss = []
                    for w in (wq, wk, wv):
                        ps = qkv_ps.tile([128, 512], F32, tag="qkv")
                        for k in range(NK):
                            nc.tensor.matmul(ps, w[:, k, :], xtt[:, k, :],
                                             start=(k == 0), stop=(k == NK - 1))
                        pss.append(ps)
                    # v: plain evacuation (fp16 round on write), on ACT
                    nc.scalar.copy(vt[:, ts], pss[2])
                    # q, k: rope
                    for ps, dst in ((pss[0], qt), (pss[1], kt)):
                        raw = rope.tile([128, 512], F16, tag="raw")
                        nc.scalar.copy(raw, ps)
                        swp = rope.tile([128, 512], F16, tag="swp")
                        for a, b2 in ((0, 32), (32, 0), (64, 96), (96, 64)):
                            nc.sync.dma_start(out=swp[a:a + 32, :],
                                              in_=raw[b2:b2 + 32, :])
                        nc.vector.tensor_mul(raw, raw, cosb[:, cs])
                        nc.vector.tensor_mul(swp, swp, sinb[:, cs])
                        nc.vector.tensor_add(dst[:, ts], raw, swp)

            if debug:
                nc.sync.dma_start(out=qt_o[:, :], in_=qt)
                nc.sync.dma_start(out=kt_o[:, :], in_=kt)
                nc.sync.dma_start(out=vt_o[:, :], in_=vt)

            # ------------- Phase 2+3: attention + out-projection --------
            with (
                tc.tile_pool(name="vp", bufs=1) as vpp,
                tc.tile_pool(name="est", bufs=4) as estp,
                tc.tile_pool(name="on", bufs=2) as onp,
                tc.tile_pool(name="inv", bufs=2) as invp,
                tc.tile_pool(name="oev", bufs=4) as oevp,
                tc.tile_pool(name="st_ps", bufs=3, space="PSUM") as st_ps,
                tc.tile_pool(name="u_ps", bufs=2, space="PSUM") as u_ps,
            ):
                for b in range(B):
                    t0 = b * T
                    # V' = [V_h | 1] token-major via DMA transpose; inner
                    # stride 80 elems = 160B keeps every transpose dst
                    # 32B-aligned (unaligned dsts corrupt silently)
                    vph = [vpp.tile([128, NB, 80], F16, tag=f"vp{h}",
                                    name=f"vp{h}_{b}")
                           for h in range(HPC)]
                    for i in range(NB):
                        blk = slice(t0 + i * 128, t0 + (i + 1) * 128)
                        for h in range(HPC):
                            nc.sync.dma_start_transpose(
                                out=vph[h][:, i, 0:64],
                                in_=vt[h * 64:(h + 1) * 64, blk])
                    for h in range(HPC):
                        nc.vector.tensor_copy(vph[h][:, :, 64], ones16)

                    on = onp.tile([128, T], F16, tag="on")
                    for h in range(HPC):
                        hp = h * 64
                        vp = vph[h]
                        pend = []

                        def normalize(u, j):
                            # bcast r (gpsimd), approx 1/r on a full tile,
                            # multiply on DVE
                            rrow = invp.tile([1, 512], F32, tag="rrow")
                            nc.vector.tensor_copy(rrow, u[64:65, :])
                            bc = invp.tile([64, 512], F32, tag="bc")
                            nc.gpsimd.partition_broadcast(bc, rrow)
                            bci = invp.tile([64, 512], F32, tag="bci")
                            nc.vector.reciprocal_approx_fast(bci, bc)
                            nc.vector.tensor_mul(
                                on[hp:hp + 64, j * 512:(j + 1) * 512],
                                u[0:64, :], bci)

                        def flush_one():
                            est0, blks, u0, nblk0, j0 = pend.pop(0)
                            for t2, i in enumerate(blks):
                                nc.tensor.matmul(
                                    u0, vp[:, i, 0:65],
                                    est0[:, t2 * 512:(t2 + 1) * 512],
                                    start=(i == 0), stop=(i == nblk0 - 1))
                            if blks[-1] == nblk0 - 1:
                                normalize(u0, j0)

                        for j in range(NJ):
                            qs = slice(t0 + j * 512, t0 + (j + 1) * 512)
                            nblk = 4 * j + 4
                            u = u_ps.tile([65, 512], F32, tag="u",
                                          name=f"u_{b}_{h}_{j}")
                            for g in range(nblk // 2):
                                st = st_ps.tile([128, 1024], F32, tag="st")
                                for t2 in range(2):
                                    i = 2 * g + t2
                                    nc.tensor.matmul(
                                        st[:, t2 * 512:(t2 + 1) * 512],
                                        kt[hp:hp + 64,
                                           t0 + i * 128: t0 + (i + 1) * 128],
                                        qt[hp:hp + 64, qs],
                                        start=True, stop=True)
                                est = estp.tile([128, 1024], F16, tag="est")
                                nc.scalar.activation(est, st, AF.Exp,
                                                     scale=float(DH) ** -0.5,
                                                     bias=ebias)
                                if 2 * g >= 4 * j:  # diagonal group
                                    mo = (2 * g - 4 * j) * 512
                                    nc.vector.tensor_mul(
                                        est, est, mask[:, mo:mo + 1024])
                                pend.append((est, (2 * g, 2 * g + 1), u,
                                             nblk, j))
                                if len(pend) > PIPE:
                                    flush_one()
                        while pend:
                            flush_one()

                    # out-projection for batch b: wo.T @ on; pairs of dout
                    # chunks share one [128,1024] psum tile from the st pool
                    for j in range(NJ):
                        ons = on[:, j * 512:(j + 1) * 512]
                        for m2 in range(NK // 2):
                            op = st_ps.tile([128, 1024], F32, tag="st")
                            for t2 in range(2):
                                m = 2 * m2 + t2
                                nc.tensor.matmul(
                                    op[:, t2 * 512:(t2 + 1) * 512],
                                    wo[:, m, :], ons, start=True, stop=True)
                            ot = oevp.tile([128, 1024], F32, tag="ot")
                            if m2 % 2 == 0:
                                nc.vector.tensor_copy(ot, op)
                            else:
                                nc.scalar.copy(ot, op)
                            for t2 in range(2):
                                m = 2 * m2 + t2
                                nc.sync.dma_start(
                                    out=out_d[m * 128:(m + 1) * 128,
                                              t0 + j * 512: t0 + (j + 1) * 512],
                                    in_=ot[:, t2 * 512:(t2 + 1) * 512])

    nc.compile()
    return nc


def _get_nc(debug=False):
    key = "ncd" if debug else "nc"
    if key not in _CACHE:
        _CACHE[key] = _build(debug)
    return _CACHE[key]


def _run(nc, in_maps, trace=False):
    from concourse.bass_utils import run_bass_kernel_spmd

    last = None
    for attempt in range(3):
        try:
            return run_bass_kernel_spmd(nc, in_maps,
                                        core_ids=list(range(N_CORES)),
                                        trace=trace)
        except Exception as e:  # transient device faults: retry
            last = e
            if "UNRECOVERABLE" not in str(e) and "UNAVAILABLE" not in str(e):
                raise
    raise last


def kernel(x, w_qkv, w_out, _trace=False, _debug=False):
    x = np.asarray(x, dtype=np.float32)
    w_qkv = np.asarray(w_qkv, dtype=np.float32)
    w_out = np.asarray(w_out, dtype=np.float32)

    xt = np.ascontiguousarray(x.reshape(BT, D).T).astype(np.float16)
    cosb, sinb, mask = _host_consts()

    in_maps = []
    for c in range(N_CORES):
        h0 = HPC * c
        cols = np.arange(h0 * DH, (h0 + HPC) * DH)
        in_maps.append({
            "xt": xt,
            "wq": np.ascontiguousarray(w_qkv[:, cols]).astype(np.float16),
            "wk": np.ascontiguousarray(w_qkv[:, D + cols]).astype(np.float16),
            "wv": np.ascontiguousarray(w_qkv[:, 2 * D + cols]).astype(np.float16),
            "wo": np.ascontiguousarray(w_out[cols, :]).astype(np.float16),
            "cosb": cosb,
            "sinb": sinb,
            "mask": mask,
        })

    nc = _get_nc(_debug)
    res = _run(nc, in_maps, trace=_trace)
    acc = np.zeros((D, BT), dtype=np.float64)
    for c in range(N_CORES):
        acc += res.results[c]["outp"]
    out = acc.T.astype(np.float32).reshape(B, T, D)
    if _debug:
        return out, res
    if _trace:
        return out, res
    return out
